# revision 1
# baseline (speedup 1.0000x reference)
"""Trainium2 Bass kernel for nn_MultiHeadAttention_75548474736720.

Linear-attention-style multi-head attention with causal prefix sums:
  qh/kh/vh = projections, ph = split_heads(p)
  A1 = elu(qh ph^T) + 1                       [t,s] per (b,h)
  U  = (tril(qh kh^T)/idx) @ A1 ; W = softmax(U)
  S2 = (tril(W A1^T)/idx) ; out = (S2 @ vh) reshaped @ wc + b

Sharding: 8 cores = (batch b in 0..1) x (head-group hg in 0..3, 4 heads each).
Each core computes its 4 heads end-to-end (wq/wk/wv column-sliced, wc
row-sliced) and returns a partial [S, Dm] output; host sums partials per batch.

All matmuls run in bf16 (f32 PSUM accumulation).  Measured end-to-end error
vs the f32 reference is ~4e-3 relative.  Key algebraic tricks:
  - exp without max-subtraction (U bounded ~|19| for this problem family)
  - softmax denominator via ACT accum_out (free with the exp pass)
  - per-row 1/(t+1) scales folded into ACT scale APs (pre-exp and final)
  - W^T / A1^T produced by PE transposes so both S*S matmuls contract K=128
"""

import sys

sys.path.insert(0, "/opt/trn_rl_repo")

import ml_dtypes
import numpy as np

import concourse.bass as bass  # noqa: F401  (registers AP machinery)
import concourse.mybir as mybir
from concourse import bacc
from concourse.tile import TileContext
from concourse.bass_utils import run_bass_kernel_spmd

F32 = mybir.dt.float32
BF16 = mybir.dt.bfloat16
ACTF = mybir.ActivationFunctionType
ALU = mybir.AluOpType
NPBF = ml_dtypes.bfloat16

B, S, DM, H = 2, 1024, 1024, 16
D = DM // H            # 64, head dim
HG = 4                 # heads per core
DL = HG * D            # 256, local dm slice
NB = S // 128          # 8 s-blocks
NORM_D = 0.125         # 1/sqrt(D)

# compact SqT layout: per s-block m, columns stored from t = 512*(m//4)
SQBASE = [0, 1024, 2048, 3072, 4096, 4608, 5120, 5632]  # total 6144

DEBUG = False


def _sq_off(m, t0):
    return SQBASE[m] + t0 - 512 * (m // 4)


def _build_program():
    nc = bacc.Bacc(None, target_bir_lowering=False)

    qT_in = nc.declare_dram_parameter("qT", [DM, S], BF16, isOutput=False)
    kT_in = nc.declare_dram_parameter("kT", [DM, S], BF16, isOutput=False)
    vT_in = nc.declare_dram_parameter("vT", [DM, S], BF16, isOutput=False)
    pT_in = nc.declare_dram_parameter("pT", [DL, S], BF16, isOutput=False)
    wq_in = nc.declare_dram_parameter("wq", [DM, DL], BF16, isOutput=False)
    wk_in = nc.declare_dram_parameter("wk", [DM, DL], BF16, isOutput=False)
    wv_in = nc.declare_dram_parameter("wv", [DM, DL], BF16, isOutput=False)
    wc_in = nc.declare_dram_parameter("wc", [DL, S], BF16, isOutput=False)
    wqb_in = nc.declare_dram_parameter("wqb", [128, 2], F32, isOutput=False)
    wkb_in = nc.declare_dram_parameter("wkb", [128, 2], F32, isOutput=False)
    wvb_in = nc.declare_dram_parameter("wvb", [1, DL], BF16, isOutput=False)
    ones_in = nc.declare_dram_parameter("ones1", [1, 128], BF16, isOutput=False)
    mask_in = nc.declare_dram_parameter("mask4", [4, 128, 512], BF16, isOutput=False)
    ident_in = nc.declare_dram_parameter("ident", [128, 128], BF16, isOutput=False)
    inv_in = nc.declare_dram_parameter("invidx", [128, NB], F32, isOutput=False)
    out_d = nc.declare_dram_parameter("out", [S, DM], F32, isOutput=True)
    dbg = {}
    if DEBUG:
        dbg["qhT"] = nc.declare_dram_parameter("d_qhT", [128, 2 * S], F32, isOutput=True)
        dbg["vh"] = nc.declare_dram_parameter("d_vh", [128, NB * DL], F32, isOutput=True)
        dbg["a1"] = nc.declare_dram_parameter("d_a1", [128, NB * S], F32, isOutput=True)
        dbg["sqT"] = nc.declare_dram_parameter("d_sqT", [128, 6144], F32, isOutput=True)
        dbg["wtT"] = nc.declare_dram_parameter("d_wtT", [128, NB * S], F32, isOutput=True)
        dbg["oT"] = nc.declare_dram_parameter("d_oT", [64, HG * S], F32, isOutput=True)
        dbg["den"] = nc.declare_dram_parameter("d_den", [128, NB], F32, isOutput=True)

    with TileContext(nc) as tc:
        with tc.tile_pool(name="persist", bufs=1) as cp, \
             tc.tile_pool(name="ppm", bufs=4, space="PSUM") as ppm, \
             tc.tile_pool(name="ppt", bufs=2, space="PSUM") as ppt:

            mask = cp.tile([128, 4, 512], BF16)
            ident = cp.tile([128, 128], BF16)
            invidx = cp.tile([128, NB], F32)
            wqb = cp.tile([128, 2], F32)
            wkb = cp.tile([128, 2], F32)
            wvb = cp.tile([1, DL], BF16)
            ones1 = cp.tile([1, 128], BF16)
            pTt = cp.tile([128, 2, S], BF16)
            qhT = cp.tile([128, 2, S], BF16)
            khT = cp.tile([128, 2, S], BF16)
            vh = cp.tile([128, NB, DL], BF16)
            oT = cp.tile([128, 2, S], BF16)
            # wc stored per head-pair: wct[:, g, :] = wc rows [g*128:(g+1)*128];
            # loaded up front so the output projection never waits on DMA
            wct = cp.tile([128, 2, S], BF16)

            # ---------------- projections ----------------
            # DMA issue on SP costs ~0.5us per descriptor, so the inputs the
            # first matmuls need go first, split 4-ways for queue parallelism;
            # constants (masks, wc, p) follow.  The v projection runs inside
            # the attention phase (interleaved with head 0's A1/SqT) so its
            # tiles live in a separate pool that outlives the q/k one.
            vp_cm = tc.tile_pool(name="vproj", bufs=1)
            vp = vp_cm.__enter__()
            wvt = vp.tile([128, NB, DL], BF16)
            vTt = vp.tile([128, NB, S], BF16)
            with tc.tile_pool(name="proj", bufs=1) as jp:
                wqt = jp.tile([128, NB, DL], BF16)
                wkt = jp.tile([128, NB, DL], BF16)
                qTt = jp.tile([128, NB, S], BF16)
                kTt = jp.tile([128, NB, S], BF16)
                for wt_, wsrc, xt_, xsrc in ((wqt, wq_in, qTt, qT_in),
                                             (wkt, wk_in, kTt, kT_in),
                                             (wvt, wv_in, vTt, vT_in)):
                    for q4 in range(4):
                        kb = 2 * q4
                        nc.sync.dma_start(
                            out=wt_[:, kb:kb + 2, :],
                            in_=wsrc[kb * 128:(kb + 2) * 128, :].rearrange(
                                "(a p) d -> p a d", p=128))
                        nc.sync.dma_start(
                            out=xt_[:, kb:kb + 2, :],
                            in_=xsrc[kb * 128:(kb + 2) * 128, :].rearrange(
                                "(a p) t -> p a t", p=128))
                    if wt_ is wqt:
                        nc.sync.dma_start(
                            out=pTt[:], in_=pT_in.rearrange("(g p) t -> p g t", p=128))
                        nc.sync.dma_start(out=wqb[:], in_=wqb_in[:])
                        nc.sync.dma_start(out=invidx[:], in_=inv_in[:])
                    elif wt_ is wkt:
                        nc.sync.dma_start(
                            out=mask[:], in_=mask_in.rearrange("r p c -> p r c"))
                        nc.sync.dma_start(out=ident[:], in_=ident_in[:])
                        nc.sync.dma_start(out=wkb[:], in_=wkb_in[:])
                    else:
                        nc.sync.dma_start(out=wvb[:], in_=wvb_in[:])
                        nc.sync.dma_start(out=ones1[:], in_=ones_in[:])
                        nc.sync.dma_start(
                            out=wct[:], in_=wc_in.rearrange("(a p) t -> p a t", p=128))

                # qhT[dm, t] = sum_c wq[c, dm] qT[c, t]  (+bias, * 1/sqrt(D))
                for wt_, xt_, dst, bias_t, scale in (
                    (wqt, qTt, qhT, wqb, NORM_D),
                    (wkt, kTt, khT, wkb, 1.0),
                ):
                    for g in range(2):
                        for n in range(2):
                            ps = ppm.tile([128, 512], F32, tag="mm", name="ps_proj")
                            for kb in range(NB):
                                nc.tensor.matmul(
                                    ps[:], wt_[:, kb, g * 128:(g + 1) * 128],
                                    xt_[:, kb, n * 512:(n + 1) * 512],
                                    start=(kb == 0), stop=(kb == NB - 1))
                            nc.scalar.activation(
                                dst[:, g, n * 512:(n + 1) * 512], ps[:],
                                ACTF.Identity, bias=bias_t[:, g:g + 1], scale=scale)

                if DEBUG:
                    nc.sync.dma_start(out=dbg["qhT"].rearrange("p (a b) -> p a b", a=2),
                                      in_=qhT[:])

            # ---------------- attention (4 heads) ----------------
            # Pair-level software pipeline: A1/SqT for head h+1 are emitted
            # between U(h) and S2(h) so the in-order PE stream always has
            # independent matmuls to run while elementwise chains drain.
            with tc.tile_pool(name="attn", bufs=2) as ap, \
                 tc.tile_pool(name="scr", bufs=2) as sp:
                st = {}

                def gen_a1_sq(h):
                    """Generator: yields after each matmul unit so A1/SqT of
                    head h can be interleaved into head h-1's S2 phase (keeps
                    the in-order PE queue fed while elementwise chains drain).

                    A1 = elu(x)+1 = min(exp(x), 1) + relu(x); exp is safe
                    unclamped (|x| <= ~8 here).  The min runs on idle GPSIMD
                    so PSUM is only held by the exp (ACT) + fused max-add
                    (DVE)."""
                    g, p0 = h // 2, (h % 2) * 64
                    a1 = ap.tile([128, NB, S], BF16, tag="a1", name="a1")
                    sqT = ap.tile([128, 6144], BF16, tag="sq", name="sqT")
                    st[h] = [a1, sqT]
                    for m in range(NB):
                        for c in range(2):
                            ps = ppm.tile([128, 512], F32, tag="a1ps", bufs=2,
                                          name="ps_a1")
                            nc.tensor.matmul(
                                ps[:], qhT[p0:p0 + 64, g, m * 128:(m + 1) * 128],
                                pTt[p0:p0 + 64, g, c * 512:(c + 1) * 512],
                                start=True, stop=True)
                            e = sp.tile([128, 512], F32, tag="e", bufs=4, name="e")
                            nc.scalar.activation(e[:], ps[:], ACTF.Exp)
                            e1 = sp.tile([128, 512], F32, tag="e1", bufs=4, name="e1")
                            nc.gpsimd.tensor_scalar_min(e1[:], e[:], 1.0)
                            nc.vector.scalar_tensor_tensor(
                                a1[:, m, c * 512:(c + 1) * 512], ps[:], 0.0, e1[:],
                                ALU.max, ALU.add)
                            yield
                    for m in range(NB):
                        for n in range(m // 4, 2):
                            ps = ppm.tile([128, 512], F32, tag="mm", name="ps_sq")
                            nc.tensor.matmul(
                                ps[:], khT[p0:p0 + 64, g, m * 128:(m + 1) * 128],
                                qhT[p0:p0 + 64, g, n * 512:(n + 1) * 512],
                                start=True, stop=True)
                            dst = sqT[:, _sq_off(m, n * 512):_sq_off(m, n * 512) + 512]
                            if n == m // 4:
                                nc.vector.tensor_tensor(dst, ps[:], mask[:, m % 4, :], ALU.mult)
                            else:
                                nc.scalar.activation(dst, ps[:], ACTF.Copy)
                            yield
                    if DEBUG and h == 0:
                        nc.sync.dma_start(
                            out=dbg["a1"].rearrange("p (a b) -> p a b", a=NB), in_=a1[:])
                        nc.sync.dma_start(out=dbg["sqT"][:, :], in_=sqT[:])

                def emit_u(h):
                    a1, sqT = st[h]
                    # U row-blocks -> exp(scale*U) -> normalize -> W^T via PE
                    # transpose.  The transposes for block i-1 are emitted
                    # after block i's matmuls so the PE stream never waits on
                    # the exp/normalize chain.
                    wtT = ap.tile([128, NB, S], BF16, tag="wtT", bufs=1, name="wtT")
                    wblks = []

                    def emit_w_transpose(i):
                        wblk = wblks[i]
                        tps = ppt.tile([128, S], BF16, tag="tp", name="tps")
                        for k in range(NB):
                            nc.tensor.transpose(
                                tps[:, k * 128:(k + 1) * 128],
                                wblk[:, k * 128:(k + 1) * 128], ident[:])
                        nc.vector.tensor_copy(
                            wtT[:, :, i * 128:(i + 1) * 128],
                            tps[:].rearrange("p (a b) -> p a b", a=NB))

                    denB = sp.tile([128, NB], F32, tag="denB", name="denB")
                    for i in range(NB):
                        wblk = sp.tile([128, S], BF16, tag="wblk", bufs=NB, name="wblk")
                        wblks.append(wblk)
                        dps = []
                        for c in range(2):
                            ps = ppm.tile([128, 512], F32, tag="mm", name="ps_u")
                            for m in range(i + 1):
                                nc.tensor.matmul(
                                    ps[:], sqT[:, _sq_off(m, i * 128):_sq_off(m, i * 128) + 128],
                                    a1[:, m, c * 512:(c + 1) * 512],
                                    start=(m == 0), stop=(m == i))
                            dp = sp.tile([128, 1], F32, tag="dp", bufs=4, name="dp")
                            nc.scalar.activation(
                                wblk[:, c * 512:(c + 1) * 512], ps[:], ACTF.Exp,
                                scale=invidx[:, i:i + 1], accum_out=dp[:])
                            dps.append(dp)
                        nc.vector.tensor_tensor(denB[:, i:i + 1], dps[0][:], dps[1][:], ALU.add)
                        if i >= 2:
                            emit_w_transpose(i - 2)
                    for i in range(NB - 2, NB):
                        emit_w_transpose(i)
                    if DEBUG and h == 0:
                        nc.sync.dma_start(out=dbg["den"], in_=denB[:])
                        nc.sync.dma_start(
                            out=dbg["wtT"].rearrange("p (a b) -> p a b", a=NB), in_=wtT[:])
                    # 1/denominator as a [1, S] row (t on the free axis) for the
                    # O^T broadcast multiply: reciprocal then a 4KB scatter DMA
                    recden = sp.tile([128, NB], F32, tag="recden", name="recden")
                    nc.vector.reciprocal(recden[:], denB[:])
                    gsc = sp.tile([128, NB], F32, tag="gsc", name="gsc")
                    nc.vector.tensor_tensor(gsc[:], recden[:], invidx[:], ALU.mult)
                    st[h].append(wtT)
                    st[h].append(gsc)

                def emit_s2(h, inter=None, tail=None):
                    def pull(k):
                        if inter is not None:
                            for _ in range(k):
                                if next(inter, "done") == "done":
                                    break
                    g, p0 = h // 2, (h % 2) * 64
                    a1, sqT, wtT, gsc = st.pop(h)
                    # S2T[s, t] = sum_j A1[s, j] W[t, j], tril(s<=t)
                    s2T = ap.tile([128, NB, S], BF16, tag="s2", bufs=1, name="s2T")
                    for n in range(2):
                        for mz, w in ((4 * n + 1, 128), (4 * n + 2, 256),
                                      (4 * n + 3, 384)):
                            # mask[:, 3, 0:384] is identically zero — zero-fill
                            # the never-computed gaps read by the O matmuls
                            nc.gpsimd.tensor_copy(
                                s2T[:, mz, n * 512:n * 512 + w], mask[:, 3, 0:w])

                    # a1T strips are produced one m ahead of the S2 matmuls
                    # that consume them
                    a1Ts = {}

                    def emit_a1t(m):
                        a1T = sp.tile([128, NB, 128], BF16, tag="a1T", bufs=4,
                                      name="a1T")
                        a1Ts[m] = a1T
                        tps = ppt.tile([128, S], BF16, tag="tp", name="tps2")
                        for k in range(NB):
                            nc.tensor.transpose(
                                tps[:, k * 128:(k + 1) * 128],
                                a1[:, m, k * 128:(k + 1) * 128], ident[:])
                        nc.scalar.activation(
                            a1T[:], tps[:].rearrange("p (a b) -> p a b", a=NB),
                            ACTF.Copy)

                    emit_a1t(0)
                    for m in range(NB):
                        if m + 1 < NB:
                            emit_a1t(m + 1)
                        a1T = a1Ts.pop(m)
                        if m % 2 == 1:
                            pull(1)
                            ps = ppm.tile([128, 128], F32, tag="mm", name="ps_s2d")
                            for k in range(NB):
                                nc.tensor.matmul(
                                    ps[:], a1T[:, k, :],
                                    wtT[:, k, m * 128:(m + 1) * 128],
                                    start=(k == 0), stop=(k == NB - 1))
                            nc.vector.tensor_tensor(
                                s2T[:, m, m * 128:(m + 1) * 128], ps[:],
                                mask[:, 0, 0:128], ALU.mult)
                        for n in range((m + 1) // 2, 4):
                            pull(1 if m < 4 else 2)
                            ps = ppm.tile([128, 256], F32, tag="mm", name="ps_s2")
                            for k in range(NB):
                                nc.tensor.matmul(
                                    ps[:], a1T[:, k, :], wtT[:, k, n * 256:(n + 1) * 256],
                                    start=(k == 0), stop=(k == NB - 1))
                            dst = s2T[:, m, n * 256:(n + 1) * 256]
                            if m % 2 == 0 and n == m // 2:
                                nc.vector.tensor_tensor(dst, ps[:], mask[:, 0, 0:256], ALU.mult)
                            else:
                                nc.vector.tensor_copy(dst, ps[:])
                        if tail is not None and m >= 2:
                            tail(m - 2, s2T, gsc)

                    st[h] = (s2T, gsc)

                oNs = {}

                def emit_o(h):
                    # O[t, d] = gsc[t] * sum_{s<=t} S2T[s, t] vh[s, d], with
                    # gsc = 1/(den*(t+1)) as a per-partition ACT scale (W was
                    # left unnormalized).  Heads h, h+1 share one oN tile
                    # (free-axis halves) so a single [128,128] PE transpose
                    # yields the stacked [d, t] layout and the output
                    # projection contracts K=128 per head-pair.
                    s2T, gsc = st.pop(h)
                    if h % 2 == 0:
                        oNs[h // 2] = sp.tile([128, NB, 128], BF16, tag="oN",
                                              bufs=4, name="oN")
                    oN = oNs[h // 2]
                    d0 = (h % 2) * 64
                    for i in range(NB):
                        ps = ppm.tile([128, 64], F32, tag="mm", name="ps_o")
                        for m in range(i + 1):
                            nc.tensor.matmul(
                                ps[:], s2T[:, m, i * 128:(i + 1) * 128],
                                vh[:, m, h * 64:(h + 1) * 64],
                                start=(m == 0), stop=(m == i))
                        nc.scalar.activation(oN[:, i, d0:d0 + 64], ps[:], ACTF.Copy,
                                             scale=gsc[:, i:i + 1])
                    if h % 2 == 1:
                        oN = oNs.pop(h // 2)
                        tps = ppm.tile([128, S], BF16, tag="mm", name="tpo")
                        for i in range(NB):
                            nc.tensor.transpose(
                                tps[:, i * 128:(i + 1) * 128], oN[:, i, :], ident[:])
                        nc.scalar.activation(
                            oT[:, h // 2, :],
                            tps[:].rearrange("p (a b) -> p a b", a=NB), ACTF.Copy)

                def emit_final_tile(i):
                    # out[t-block i, :] = sum_g oT_g^T wc_g (all scales already
                    # folded into oT)
                    for c in range(2):
                        ftag = "mm" if (i + c) % 2 == 0 else "a1ps"
                        ps = ppm.tile([128, 512], F32, tag=ftag, name="ps_fin",
                                      bufs=(4 if ftag == "mm" else 2))
                        for g2 in range(2):
                            nc.tensor.matmul(
                                ps[:], oT[:, g2, i * 128:(i + 1) * 128],
                                wct[:, g2, c * 512:(c + 1) * 512],
                                start=(g2 == 0), stop=(g2 == 1))
                        ot = sp.tile([128, 512], F32, tag="ot", bufs=6, name="ot")
                        if (i + c) % 2 == 0:
                            nc.scalar.activation(ot[:], ps[:], ACTF.Copy)
                        else:
                            nc.vector.tensor_copy(ot[:], ps[:])
                        nc.sync.dma_start(
                            out=out_d[i * 128:(i + 1) * 128, c * 512:(c + 1) * 512],
                            in_=ot[:])

                # vh[s, d] = sum_c vT[c, s] wv[c, d] + wv_b[d], interleaved
                # with head 0's A1/SqT so PE has work while vT streams in
                gen0 = gen_a1_sq(0)
                for m in range(NB):
                    ps = ppm.tile([128, DL], F32, tag="mm", name="ps_vh")
                    for kb in range(NB):
                        nc.tensor.matmul(
                            ps[:], vTt[:, kb, m * 128:(m + 1) * 128], wvt[:, kb, :],
                            start=(kb == 0), stop=False)
                    nc.tensor.matmul(ps[:], ones1[:], wvb[:], start=False, stop=True)
                    nc.scalar.activation(vh[:, m, :], ps[:], ACTF.Copy)
                    for _ in range(3):
                        if next(gen0, "done") == "done":
                            break
                for _ in gen0:
                    pass
                if DEBUG:
                    nc.sync.dma_start(out=dbg["vh"].rearrange("p (a b) -> p a b", a=NB),
                                      in_=vh[:])
                for h in range(HG):
                    emit_u(h)
                    if h >= 1:
                        emit_o(h - 1)
                    gen = gen_a1_sq(h + 1) if h + 1 < HG else None
                    emit_s2(h, inter=gen)
                    if gen is not None:
                        for _ in gen:
                            pass
                emit_o(HG - 1)
                for i in range(NB):
                    emit_final_tile(i)

            if DEBUG:
                nc.sync.dma_start(
                    out=dbg["oT"].rearrange("p (a b) -> p a b", a=HG), in_=oT[:])

            vp_cm.__exit__(None, None, None)

    nc.finalize()
    return nc


_CACHE = {}


def _get_program():
    if "nc" not in _CACHE:
        _CACHE["nc"] = _build_program()
    return _CACHE["nc"]


def _consts():
    if "consts" not in _CACHE:
        p_ = np.arange(128, dtype=np.float32)[:, None]
        c_ = np.arange(512, dtype=np.float32)[None, :]
        mask4 = np.stack(
            [(p_ + 128.0 * r <= c_) for r in range(4)]).astype(NPBF)
        ident = np.eye(128, dtype=np.float32).astype(NPBF)
        blk = np.arange(NB, dtype=np.float32)[None, :]
        invidx = (1.0 / (blk * 128.0 + p_ + 1.0)).astype(np.float32)
        ones1 = np.ones((1, 128), NPBF)
        _CACHE["consts"] = (mask4, ident, invidx, ones1)
    return _CACHE["consts"]


PROFILE = False
LAST_RESULTS = None


def kernel(v, k, q, p, wq_k, wq_b, wk_k, wk_b, wv_k, wv_b, wc_k, wc_b):
    global LAST_RESULTS
    nc = _get_program()
    mask4, ident, invidx, ones1 = _consts()

    qT = [np.ascontiguousarray(q[b].T).astype(NPBF) for b in range(B)]
    kT = [np.ascontiguousarray(k[b].T).astype(NPBF) for b in range(B)]
    vT = [np.ascontiguousarray(v[b].T).astype(NPBF) for b in range(B)]
    pT = [np.ascontiguousarray(p[b].T).astype(NPBF) for b in range(B)]
    wqc = wq_k.astype(NPBF)
    wkc = wk_k.astype(NPBF)
    wvc = wv_k.astype(NPBF)
    wcc = wc_k.astype(NPBF)

    in_maps = []
    for c in range(8):
        b, hg = c // 4, c % 4
        c0 = hg * DL
        wqb = np.ascontiguousarray(
            (wq_b[c0:c0 + DL].reshape(2, 128).T * NORM_D).astype(np.float32))
        wkb = np.ascontiguousarray(wk_b[c0:c0 + DL].reshape(2, 128).T.astype(np.float32))
        in_maps.append({
            "qT": qT[b], "kT": kT[b], "vT": vT[b],
            "pT": np.ascontiguousarray(pT[b][c0:c0 + DL]),
            "wq": np.ascontiguousarray(wqc[:, c0:c0 + DL]),
            "wk": np.ascontiguousarray(wkc[:, c0:c0 + DL]),
            "wv": np.ascontiguousarray(wvc[:, c0:c0 + DL]),
            "wc": np.ascontiguousarray(wcc[c0:c0 + DL, :]),
            "wqb": wqb, "wkb": wkb,
            "wvb": np.ascontiguousarray(wv_b[c0:c0 + DL].reshape(1, DL).astype(NPBF)),
            "ones1": ones1, "mask4": mask4, "ident": ident, "invidx": invidx,
        })

    res = run_bass_kernel_spmd(
        nc, in_maps, core_ids=list(range(8)), trace=PROFILE)
    LAST_RESULTS = res

    out = np.zeros((B, S, DM), np.float32)
    for c in range(8):
        out[c // 4] += res.results[c]["out"]
    out += wc_b[None, None, :].astype(np.float32)
    return out



# revision 8
# speedup vs baseline: 1.0507x; 1.0507x over previous
"""Trainium2 Bass kernel for nn_MultiHeadAttention_75548474736720.

Linear-attention-style multi-head attention with causal prefix sums:
  qh/kh/vh = projections, ph = split_heads(p)
  A1 = elu(qh ph^T) + 1                       [t,s] per (b,h)
  U  = (tril(qh kh^T)/idx) @ A1 ; W = softmax(U)
  out[t] = (1/(t+1)) sum_{s<=t} (W[t]·A1[s]) vh[s] ; reshape @ wc + b

Sharding: 8 cores = (batch b in 0..1) x (head-group hg in 0..3, 4 heads each).
Each core computes its 4 heads end-to-end (wq/wk/wv column-sliced, wc
row-sliced) and returns a partial [S, Dm] output; host sums partials per batch.

All matmuls run in bf16 (f32 PSUM accumulation).  Key algebraic tricks:
  - exp without max-subtraction (U bounded ~|19| for this problem family)
  - softmax denominator via ACT accum_out (free with the exp pass)
  - per-row 1/(t+1) scales folded into ACT scale APs (pre-exp and at oN)
  - W^T / A1^T produced by PE transposes so the S*S matmul contracts K=128
  - second prefix sum via a running accumulator C[j,d] = sum_{s<t0} A1[s,j]
    vh[s,d] per head: out-block i = W-block @ C + tril(W A1_i^T) @ vh_i,
    which is O(S*S*D) instead of O(S*S*S) for the explicit S2 matrix
"""

import sys

sys.path.insert(0, "/opt/trn_rl_repo")

import ml_dtypes
import numpy as np

import concourse.bass as bass  # noqa: F401  (registers AP machinery)
import concourse.mybir as mybir
from concourse import bacc
from concourse.tile import TileContext
from concourse.bass_utils import run_bass_kernel_spmd

F32 = mybir.dt.float32
BF16 = mybir.dt.bfloat16
ACTF = mybir.ActivationFunctionType
ALU = mybir.AluOpType
NPBF = ml_dtypes.bfloat16

B, S, DM, H = 2, 1024, 1024, 16
D = DM // H            # 64, head dim
HG = 4                 # heads per core
DL = HG * D            # 256, local dm slice
NB = S // 128          # 8 s-blocks
NORM_D = 0.125         # 1/sqrt(D)

# compact SqT layout: per s-block m, columns stored from t = 512*(m//4)
SQBASE = [0, 1024, 2048, 3072, 4096, 4608, 5120, 5632]  # total 6144

DEBUG = False


def _sq_off(m, t0):
    return SQBASE[m] + t0 - 512 * (m // 4)


def _build_program():
    nc = bacc.Bacc(None, target_bir_lowering=False)

    qT_in = nc.declare_dram_parameter("qT", [DM, S], BF16, isOutput=False)
    kT_in = nc.declare_dram_parameter("kT", [DM, S], BF16, isOutput=False)
    vT_in = nc.declare_dram_parameter("vT", [DM, S], BF16, isOutput=False)
    pT_in = nc.declare_dram_parameter("pT", [DL, S], BF16, isOutput=False)
    wq_in = nc.declare_dram_parameter("wq", [DM, DL], BF16, isOutput=False)
    wk_in = nc.declare_dram_parameter("wk", [DM, DL], BF16, isOutput=False)
    wv_in = nc.declare_dram_parameter("wv", [DM, DL], BF16, isOutput=False)
    wc_in = nc.declare_dram_parameter("wc", [DL, S], BF16, isOutput=False)
    wqb_in = nc.declare_dram_parameter("wqb", [128, 2], F32, isOutput=False)
    wkb_in = nc.declare_dram_parameter("wkb", [128, 2], F32, isOutput=False)
    wvb_in = nc.declare_dram_parameter("wvb", [1, DL], BF16, isOutput=False)
    ones_in = nc.declare_dram_parameter("ones1", [1, 128], BF16, isOutput=False)
    mask_in = nc.declare_dram_parameter("mask4", [4, 128, 512], BF16, isOutput=False)
    ident_in = nc.declare_dram_parameter("ident", [128, 128], BF16, isOutput=False)
    inv_in = nc.declare_dram_parameter("invidx", [128, NB], F32, isOutput=False)
    out_d = nc.declare_dram_parameter("out", [S, DM], F32, isOutput=True)
    dbg = {}
    if DEBUG:
        dbg["qhT"] = nc.declare_dram_parameter("d_qhT", [128, 2 * S], F32, isOutput=True)
        dbg["vh"] = nc.declare_dram_parameter("d_vh", [128, NB * DL], F32, isOutput=True)
        dbg["a1"] = nc.declare_dram_parameter("d_a1", [128, NB * S], F32, isOutput=True)
        dbg["sqT"] = nc.declare_dram_parameter("d_sqT", [128, 6144], F32, isOutput=True)
        dbg["wtT"] = nc.declare_dram_parameter("d_wtT", [128, NB * S], F32, isOutput=True)
        dbg["oT"] = nc.declare_dram_parameter("d_oT", [64, HG * S], F32, isOutput=True)
        dbg["den"] = nc.declare_dram_parameter("d_den", [128, NB], F32, isOutput=True)

    with TileContext(nc) as tc:
        with tc.tile_pool(name="persist", bufs=1) as cp, \
             tc.tile_pool(name="ppm", bufs=3, space="PSUM") as ppm, \
             tc.tile_pool(name="ppt", bufs=2, space="PSUM") as ppt:

            mask = cp.tile([128, 4, 512], BF16)
            ident = cp.tile([128, 128], BF16)
            invidx = cp.tile([128, NB], F32)
            wqb = cp.tile([128, 2], F32)
            wkb = cp.tile([128, 2], F32)
            wvb = cp.tile([1, DL], BF16)
            ones1 = cp.tile([1, 128], BF16)
            pTt = cp.tile([128, 2, S], BF16)
            qhT = cp.tile([128, 2, S], BF16)
            khT = cp.tile([128, 2, S], BF16)
            vh = cp.tile([128, NB, DL], BF16)
            oT = cp.tile([128, 2, S], BF16)
            # wc stored per head-pair: wct[:, g, :] = wc rows [g*128:(g+1)*128];
            # loaded up front so the output projection never waits on DMA
            wct = cp.tile([128, 2, S], BF16)

            # ---------------- projections ----------------
            # DMA issue on SP costs ~0.5us per descriptor, so the inputs the
            # first matmuls need go first, split 4-ways for queue parallelism;
            # constants (masks, wc, p) follow.  The v projection runs inside
            # the attention phase (interleaved with head 0's A1/SqT) so its
            # tiles live in a separate pool that outlives the q/k one.
            vp_cm = tc.tile_pool(name="vproj", bufs=1)
            vp = vp_cm.__enter__()
            wvt = vp.tile([128, NB, DL], BF16)
            vTt = vp.tile([128, NB, S], BF16)
            with tc.tile_pool(name="proj", bufs=1) as jp:
                wqt = jp.tile([128, NB, DL], BF16)
                wkt = jp.tile([128, NB, DL], BF16)
                qTt = jp.tile([128, NB, S], BF16)
                kTt = jp.tile([128, NB, S], BF16)
                for wt_, wsrc, xt_, xsrc in ((wqt, wq_in, qTt, qT_in),
                                             (wkt, wk_in, kTt, kT_in),
                                             (wvt, wv_in, vTt, vT_in)):
                    for q4 in range(4):
                        kb = 2 * q4
                        nc.sync.dma_start(
                            out=wt_[:, kb:kb + 2, :],
                            in_=wsrc[kb * 128:(kb + 2) * 128, :].rearrange(
                                "(a p) d -> p a d", p=128))
                        nc.sync.dma_start(
                            out=xt_[:, kb:kb + 2, :],
                            in_=xsrc[kb * 128:(kb + 2) * 128, :].rearrange(
                                "(a p) t -> p a t", p=128))
                    if wt_ is wqt:
                        nc.sync.dma_start(
                            out=pTt[:], in_=pT_in.rearrange("(g p) t -> p g t", p=128))
                        nc.sync.dma_start(out=wqb[:], in_=wqb_in[:])
                        nc.sync.dma_start(out=invidx[:], in_=inv_in[:])
                    elif wt_ is wkt:
                        nc.sync.dma_start(
                            out=mask[:], in_=mask_in.rearrange("r p c -> p r c"))
                        nc.sync.dma_start(out=ident[:], in_=ident_in[:])
                        nc.sync.dma_start(out=wkb[:], in_=wkb_in[:])
                    else:
                        nc.sync.dma_start(out=wvb[:], in_=wvb_in[:])
                        nc.sync.dma_start(out=ones1[:], in_=ones_in[:])
                        nc.sync.dma_start(
                            out=wct[:], in_=wc_in.rearrange("(a p) t -> p a t", p=128))

                # qhT[dm, t] = sum_c wq[c, dm] qT[c, t]  (+bias, * 1/sqrt(D))
                for wt_, xt_, dst, bias_t, scale in (
                    (wqt, qTt, qhT, wqb, NORM_D),
                    (wkt, kTt, khT, wkb, 1.0),
                ):
                    for g in range(2):
                        for n in range(2):
                            ps = ppm.tile([128, 512], F32, tag="mm", name="ps_proj")
                            for kb in range(NB):
                                nc.tensor.matmul(
                                    ps[:], wt_[:, kb, g * 128:(g + 1) * 128],
                                    xt_[:, kb, n * 512:(n + 1) * 512],
                                    start=(kb == 0), stop=(kb == NB - 1))
                            nc.scalar.activation(
                                dst[:, g, n * 512:(n + 1) * 512], ps[:],
                                ACTF.Identity, bias=bias_t[:, g:g + 1], scale=scale)

                if DEBUG:
                    nc.sync.dma_start(out=dbg["qhT"].rearrange("p (a b) -> p a b", a=2),
                                      in_=qhT[:])

            # ---------------- attention (4 heads) ----------------
            # Pair-level software pipeline: A1/SqT for head h+1 are emitted
            # between U(h) and S2(h) so the in-order PE stream always has
            # independent matmuls to run while elementwise chains drain.
            with tc.tile_pool(name="attn", bufs=2) as ap, \
                 tc.tile_pool(name="scr", bufs=2) as sp:
                st = {}

                def gen_a1_sq(h):
                    """Generator: yields after each matmul unit so A1/SqT of
                    head h can be interleaved into head h-1's S2 phase (keeps
                    the in-order PE queue fed while elementwise chains drain).

                    A1 = elu(x)+1 = min(exp(x), 1) + relu(x); exp is safe
                    unclamped (|x| <= ~8 here).  The min runs on idle GPSIMD
                    so PSUM is only held by the exp (ACT) + fused max-add
                    (DVE)."""
                    g, p0 = h // 2, (h % 2) * 64
                    a1 = ap.tile([128, NB, S], BF16, tag="a1", name="a1")
                    sqT = ap.tile([128, 6144], BF16, tag="sq", name="sqT")
                    st[h] = [a1, sqT]
                    for m in range(NB):
                        for c in range(2):
                            ps = ppm.tile([128, 512], F32, tag="mm", name="ps_a1")
                            nc.tensor.matmul(
                                ps[:], qhT[p0:p0 + 64, g, m * 128:(m + 1) * 128],
                                pTt[p0:p0 + 64, g, c * 512:(c + 1) * 512],
                                start=True, stop=True)
                            e = sp.tile([128, 512], F32, tag="e", bufs=4, name="e")
                            nc.scalar.activation(e[:], ps[:], ACTF.Exp)
                            e1 = sp.tile([128, 512], F32, tag="e1", bufs=4, name="e1")
                            nc.gpsimd.tensor_scalar_min(e1[:], e[:], 1.0)
                            nc.vector.scalar_tensor_tensor(
                                a1[:, m, c * 512:(c + 1) * 512], ps[:], 0.0, e1[:],
                                ALU.max, ALU.add)
                            yield
                    for m in range(NB):
                        for n in range(m // 4, 2):
                            ps = ppm.tile([128, 512], F32, tag="mm", name="ps_sq")
                            nc.tensor.matmul(
                                ps[:], khT[p0:p0 + 64, g, m * 128:(m + 1) * 128],
                                qhT[p0:p0 + 64, g, n * 512:(n + 1) * 512],
                                start=True, stop=True)
                            dst = sqT[:, _sq_off(m, n * 512):_sq_off(m, n * 512) + 512]
                            if n == m // 4:
                                nc.vector.tensor_tensor(dst, ps[:], mask[:, m % 4, :], ALU.mult)
                            else:
                                nc.scalar.activation(dst, ps[:], ACTF.Copy)
                            yield
                    if DEBUG and h == 0:
                        nc.sync.dma_start(
                            out=dbg["a1"].rearrange("p (a b) -> p a b", a=NB), in_=a1[:])
                        nc.sync.dma_start(out=dbg["sqT"][:, :], in_=sqT[:])

                def emit_u(h):
                    a1, sqT = st[h]
                    # U row-blocks -> exp(scale*U) -> normalize -> W^T via PE
                    # transpose.  The transposes for block i-1 are emitted
                    # after block i's matmuls so the PE stream never waits on
                    # the exp/normalize chain.
                    wtT = ap.tile([128, NB, S], BF16, tag="wtT", bufs=1, name="wtT")
                    wblks = []

                    def emit_w_transpose(i):
                        wblk = wblks[i]
                        tps = ppt.tile([128, S], BF16, tag="tp", name="tps")
                        for k in range(NB):
                            nc.tensor.transpose(
                                tps[:, k * 128:(k + 1) * 128],
                                wblk[:, k * 128:(k + 1) * 128], ident[:])
                        nc.vector.tensor_copy(
                            wtT[:, :, i * 128:(i + 1) * 128],
                            tps[:].rearrange("p (a b) -> p a b", a=NB))

                    denB = sp.tile([128, NB], F32, tag="denB", name="denB")
                    for i in range(NB):
                        wblk = sp.tile([128, S], BF16, tag="wblk", bufs=NB, name="wblk")
                        wblks.append(wblk)
                        dps = []
                        for c in range(2):
                            ps = ppm.tile([128, 512], F32, tag="mm", name="ps_u")
                            for m in range(i + 1):
                                nc.tensor.matmul(
                                    ps[:], sqT[:, _sq_off(m, i * 128):_sq_off(m, i * 128) + 128],
                                    a1[:, m, c * 512:(c + 1) * 512],
                                    start=(m == 0), stop=(m == i))
                            dp = sp.tile([128, 1], F32, tag="dp", bufs=4, name="dp")
                            nc.scalar.activation(
                                wblk[:, c * 512:(c + 1) * 512], ps[:], ACTF.Exp,
                                scale=invidx[:, i:i + 1], accum_out=dp[:])
                            dps.append(dp)
                        nc.vector.tensor_tensor(denB[:, i:i + 1], dps[0][:], dps[1][:], ALU.add)
                        if i >= 2:
                            emit_w_transpose(i - 2)
                    for i in range(NB - 2, NB):
                        emit_w_transpose(i)
                    if DEBUG and h == 0:
                        nc.sync.dma_start(out=dbg["den"], in_=denB[:])
                        nc.sync.dma_start(
                            out=dbg["wtT"].rearrange("p (a b) -> p a b", a=NB), in_=wtT[:])
                    # 1/denominator as a [1, S] row (t on the free axis) for the
                    # O^T broadcast multiply: reciprocal then a 4KB scatter DMA
                    recden = sp.tile([128, NB], F32, tag="recden", name="recden")
                    nc.vector.reciprocal(recden[:], denB[:])
                    gsc = sp.tile([128, NB], F32, tag="gsc", name="gsc")
                    nc.vector.tensor_tensor(gsc[:], recden[:], invidx[:], ALU.mult)
                    st[h].append(wtT)
                    st[h].append(gsc)

                oNs = {}

                def emit_c(h, inter=None):
                    """out[t-block i] = gsc[t] * (W-blk_i @ C^(i) +
                    tril(W_i A1_i^T) @ vh_i) with the running accumulator
                    C^(i)[j, d] = sum_{s < 128i} A1[s, j] vh[s, d] carried in
                    bf16 (one DVE add per block).  gsc = 1/(den*(t+1)) as a
                    per-partition ACT scale (W was left unnormalized).  Heads
                    h, h+1 share one oN tile (free-axis halves) so a single
                    [128,128] PE transpose per t-block yields the stacked
                    [d, t] layout and the output projection contracts K=128
                    per head-pair."""
                    def pull(k):
                        if inter is not None:
                            for _ in range(k):
                                if next(inter, "done") == "done":
                                    break
                    a1, sqT, wtT, gsc = st.pop(h)
                    d0h = h * 64
                    if h % 2 == 0:
                        oNs[h // 2] = sp.tile([128, NB, 128], BF16, tag="oN",
                                              bufs=2, name="oN")
                    oN = oNs[h // 2]
                    d0 = (h % 2) * 64

                    # a1T strips are produced one block ahead of the S2-diag
                    # matmuls that consume them; the PSUM->SBUF copies
                    # alternate between ACT and DVE to balance engine load
                    a1Ts = {}

                    def emit_a1t(m):
                        a1T = sp.tile([128, NB, 128], BF16, tag="a1T", bufs=4,
                                      name="a1T")
                        a1Ts[m] = a1T
                        tps = ppt.tile([128, S], BF16, tag="tp", name="tps2")
                        for k in range(NB):
                            nc.tensor.transpose(
                                tps[:, k * 128:(k + 1) * 128],
                                a1[:, m, k * 128:(k + 1) * 128], ident[:])
                        tv = tps[:].rearrange("p (a b) -> p a b", a=NB)
                        if m % 2 == 0:
                            nc.scalar.activation(a1T[:], tv, ACTF.Copy)
                        else:
                            nc.vector.tensor_copy(a1T[:], tv)

                    cprev = None
                    emit_a1t(0)
                    for i in range(NB):
                        if i + 1 < NB:
                            emit_a1t(i + 1)
                        a1T = a1Ts.pop(i)
                        # S2-diagonal block: S2dT[s, t] = sum_j A1[s,j] W[t,j]
                        psd = ppm.tile([128, 128], F32, tag="cd", bufs=2,
                                       name="ps_s2d")
                        for k in range(NB):
                            nc.tensor.matmul(
                                psd[:], a1T[:, k, :],
                                wtT[:, k, i * 128:(i + 1) * 128],
                                start=(k == 0), stop=(k == NB - 1))
                        pull(1)
                        s2dT = sp.tile([128, 128], BF16, tag="s2d", bufs=2,
                                       name="s2dT")
                        nc.vector.tensor_tensor(s2dT[:], psd[:],
                                                mask[:, 0, 0:128], ALU.mult)
                        # out-block i: prefix part via C, then the diag part
                        pso = ppm.tile([128, 64], F32, tag="cd", bufs=2,
                                       name="ps_o")
                        if i > 0:
                            for k in range(NB):
                                nc.tensor.matmul(
                                    pso[:], wtT[:, k, i * 128:(i + 1) * 128],
                                    cprev[:, k, :],
                                    start=(k == 0), stop=False)
                        pull(1)
                        nc.tensor.matmul(pso[:], s2dT[:],
                                         vh[:, i, d0h:d0h + 64],
                                         start=(i == 0), stop=True)
                        nc.scalar.activation(oN[:, i, d0:d0 + 64], pso[:],
                                             ACTF.Copy, scale=gsc[:, i:i + 1])
                        # C update: C^(i+1) = C^(i) + A1_i^T @ vh_i
                        if i + 1 < NB:
                            psc = ppm.tile([128, 512], F32, tag="cupd", bufs=1,
                                           name="ps_cu")
                            for k in range(NB):
                                nc.tensor.matmul(
                                    psc[:, k * 64:(k + 1) * 64],
                                    a1[:, i, k * 128:(k + 1) * 128],
                                    vh[:, i, d0h:d0h + 64],
                                    start=True, stop=True)
                            pull(1)
                            cnew = sp.tile([128, NB, 64], BF16, tag="C",
                                           bufs=3, name="C")
                            pv = psc[:].rearrange("p (a b) -> p a b", a=NB)
                            if cprev is None:
                                nc.vector.tensor_copy(cnew[:], pv)
                            else:
                                nc.vector.tensor_tensor(cnew[:], cprev[:], pv,
                                                        ALU.add)
                            cprev = cnew
                        pull(1)
                    if h % 2 == 1:
                        oNp = oNs.pop(h // 2)
                        tps = ppt.tile([128, S], BF16, tag="tp", name="tpo")
                        for i in range(NB):
                            nc.tensor.transpose(
                                tps[:, i * 128:(i + 1) * 128], oNp[:, i, :],
                                ident[:])
                        nc.scalar.activation(
                            oT[:, h // 2, :],
                            tps[:].rearrange("p (a b) -> p a b", a=NB),
                            ACTF.Copy)

                def emit_final_tile(i):
                    # out[t-block i, :] = sum_g oT_g^T wc_g (all scales already
                    # folded into oT)
                    for c in range(2):
                        ps = ppm.tile([128, 512], F32, tag="mm", name="ps_fin")
                        for g2 in range(2):
                            nc.tensor.matmul(
                                ps[:], oT[:, g2, i * 128:(i + 1) * 128],
                                wct[:, g2, c * 512:(c + 1) * 512],
                                start=(g2 == 0), stop=(g2 == 1))
                        ot = sp.tile([128, 512], F32, tag="ot", bufs=6, name="ot")
                        if (i + c) % 2 == 0:
                            nc.scalar.activation(ot[:], ps[:], ACTF.Copy)
                        else:
                            nc.vector.tensor_copy(ot[:], ps[:])
                        nc.sync.dma_start(
                            out=out_d[i * 128:(i + 1) * 128, c * 512:(c + 1) * 512],
                            in_=ot[:])

                # vh[s, d] = sum_c vT[c, s] wv[c, d] + wv_b[d], interleaved
                # with head 0's A1/SqT so PE has work while vT streams in
                gen0 = gen_a1_sq(0)
                for m in range(NB):
                    ps = ppm.tile([128, DL], F32, tag="mm", name="ps_vh")
                    for kb in range(NB):
                        nc.tensor.matmul(
                            ps[:], vTt[:, kb, m * 128:(m + 1) * 128], wvt[:, kb, :],
                            start=(kb == 0), stop=False)
                    nc.tensor.matmul(ps[:], ones1[:], wvb[:], start=False, stop=True)
                    nc.scalar.activation(vh[:, m, :], ps[:], ACTF.Copy)
                    for _ in range(3):
                        if next(gen0, "done") == "done":
                            break
                for _ in gen0:
                    pass
                if DEBUG:
                    nc.sync.dma_start(out=dbg["vh"].rearrange("p (a b) -> p a b", a=NB),
                                      in_=vh[:])
                for h in range(HG):
                    emit_u(h)
                    gen = gen_a1_sq(h + 1) if h + 1 < HG else None
                    emit_c(h, inter=gen)
                    if gen is not None:
                        for _ in gen:
                            pass
                for i in range(NB):
                    emit_final_tile(i)

            if DEBUG:
                nc.sync.dma_start(
                    out=dbg["oT"].rearrange("p (a b) -> p a b", a=HG), in_=oT[:])

            vp_cm.__exit__(None, None, None)

    nc.finalize()
    return nc


_CACHE = {}


def _get_program():
    if "nc" not in _CACHE:
        _CACHE["nc"] = _build_program()
    return _CACHE["nc"]


def _consts():
    if "consts" not in _CACHE:
        p_ = np.arange(128, dtype=np.float32)[:, None]
        c_ = np.arange(512, dtype=np.float32)[None, :]
        mask4 = np.stack(
            [(p_ + 128.0 * r <= c_) for r in range(4)]).astype(NPBF)
        ident = np.eye(128, dtype=np.float32).astype(NPBF)
        blk = np.arange(NB, dtype=np.float32)[None, :]
        invidx = (1.0 / (blk * 128.0 + p_ + 1.0)).astype(np.float32)
        ones1 = np.ones((1, 128), NPBF)
        _CACHE["consts"] = (mask4, ident, invidx, ones1)
    return _CACHE["consts"]


PROFILE = False
LAST_RESULTS = None


def kernel(v, k, q, p, wq_k, wq_b, wk_k, wk_b, wv_k, wv_b, wc_k, wc_b):
    global LAST_RESULTS
    nc = _get_program()
    mask4, ident, invidx, ones1 = _consts()

    qT = [np.ascontiguousarray(q[b].T).astype(NPBF) for b in range(B)]
    kT = [np.ascontiguousarray(k[b].T).astype(NPBF) for b in range(B)]
    vT = [np.ascontiguousarray(v[b].T).astype(NPBF) for b in range(B)]
    pT = [np.ascontiguousarray(p[b].T).astype(NPBF) for b in range(B)]
    wqc = wq_k.astype(NPBF)
    wkc = wk_k.astype(NPBF)
    wvc = wv_k.astype(NPBF)
    wcc = wc_k.astype(NPBF)

    in_maps = []
    for c in range(8):
        b, hg = c // 4, c % 4
        c0 = hg * DL
        wqb = np.ascontiguousarray(
            (wq_b[c0:c0 + DL].reshape(2, 128).T * NORM_D).astype(np.float32))
        wkb = np.ascontiguousarray(wk_b[c0:c0 + DL].reshape(2, 128).T.astype(np.float32))
        in_maps.append({
            "qT": qT[b], "kT": kT[b], "vT": vT[b],
            "pT": np.ascontiguousarray(pT[b][c0:c0 + DL]),
            "wq": np.ascontiguousarray(wqc[:, c0:c0 + DL]),
            "wk": np.ascontiguousarray(wkc[:, c0:c0 + DL]),
            "wv": np.ascontiguousarray(wvc[:, c0:c0 + DL]),
            "wc": np.ascontiguousarray(wcc[c0:c0 + DL, :]),
            "wqb": wqb, "wkb": wkb,
            "wvb": np.ascontiguousarray(wv_b[c0:c0 + DL].reshape(1, DL).astype(NPBF)),
            "ones1": ones1, "mask4": mask4, "ident": ident, "invidx": invidx,
        })

    res = run_bass_kernel_spmd(
        nc, in_maps, core_ids=list(range(8)), trace=PROFILE)
    LAST_RESULTS = res

    out = np.zeros((B, S, DM), np.float32)
    for c in range(8):
        out[c // 4] += res.results[c]["out"]
    out += wc_b[None, None, :].astype(np.float32)
    return out



# revision 14
# speedup vs baseline: 1.0783x; 1.0262x over previous
"""Trainium2 Bass kernel for nn_MultiHeadAttention_75548474736720.

Linear-attention-style multi-head attention with causal prefix sums:
  qh/kh/vh = projections, ph = split_heads(p)
  A1 = elu(qh ph^T) + 1                       [t,s] per (b,h)
  U  = (tril(qh kh^T)/idx) @ A1 ; W = softmax(U)
  out[t] = (1/(t+1)) sum_{s<=t} (W[t]·A1[s]) vh[s] ; reshape @ wc + b

Sharding: 8 cores = (batch b in 0..1) x (head-group hg in 0..3, 4 heads each).
Each core computes its 4 heads end-to-end (wq/wk/wv column-sliced, wc
row-sliced) and returns a partial [S, Dm] output; host sums partials per batch.

All matmuls run in bf16 (f32 PSUM accumulation).  Key algebraic tricks:
  - exp without max-subtraction (U bounded ~|19| for this problem family)
  - softmax denominator via ACT accum_out (free with the exp pass)
  - per-row 1/(t+1) scales folded into ACT scale APs (pre-exp and at oN)
  - W^T / A1^T produced by PE transposes so the S*S matmul contracts K=128
  - second prefix sum via a running accumulator C[j,d] = sum_{s<t0} A1[s,j]
    vh[s,d] per head: out-block i = W-block @ C + tril(W A1_i^T) @ vh_i,
    which is O(S*S*D) instead of O(S*S*S) for the explicit S2 matrix
"""

import sys

sys.path.insert(0, "/opt/trn_rl_repo")

import ml_dtypes
import numpy as np

import concourse.bass as bass  # noqa: F401  (registers AP machinery)
import concourse.mybir as mybir
from concourse import bacc
from concourse.tile import TileContext
from concourse.bass_utils import run_bass_kernel_spmd

F32 = mybir.dt.float32
BF16 = mybir.dt.bfloat16
ACTF = mybir.ActivationFunctionType
ALU = mybir.AluOpType
NPBF = ml_dtypes.bfloat16

B, S, DM, H = 2, 1024, 1024, 16
D = DM // H            # 64, head dim
HG = 4                 # heads per core
DL = HG * D            # 256, local dm slice
NB = S // 128          # 8 s-blocks
NORM_D = 0.125         # 1/sqrt(D)

# compact SqT layout: per s-block m, columns stored from t = 512*(m//4)
SQBASE = [0, 1024, 2048, 3072, 4096, 4608, 5120, 5632]  # total 6144

DEBUG = False


def _sq_off(m, t0):
    return SQBASE[m] + t0 - 512 * (m // 4)


def _build_program():
    nc = bacc.Bacc(None, target_bir_lowering=False)

    qT_in = nc.declare_dram_parameter("qT", [DM, S], BF16, isOutput=False)
    kT_in = nc.declare_dram_parameter("kT", [DM, S], BF16, isOutput=False)
    vT_in = nc.declare_dram_parameter("vT", [DM, S], BF16, isOutput=False)
    pT_in = nc.declare_dram_parameter("pT", [DL, S], BF16, isOutput=False)
    wq_in = nc.declare_dram_parameter("wq", [DM, DL], BF16, isOutput=False)
    wk_in = nc.declare_dram_parameter("wk", [DM, DL], BF16, isOutput=False)
    wv_in = nc.declare_dram_parameter("wv", [DM, DL], BF16, isOutput=False)
    wc_in = nc.declare_dram_parameter("wc", [DL, S], BF16, isOutput=False)
    wqb_in = nc.declare_dram_parameter("wqb", [128, 2], F32, isOutput=False)
    wkb_in = nc.declare_dram_parameter("wkb", [128, 2], F32, isOutput=False)
    wvb_in = nc.declare_dram_parameter("wvb", [1, DL], BF16, isOutput=False)
    ones_in = nc.declare_dram_parameter("ones1", [1, 128], BF16, isOutput=False)
    mask_in = nc.declare_dram_parameter("mask4", [4, 128, 512], BF16, isOutput=False)
    ident_in = nc.declare_dram_parameter("ident", [128, 128], BF16, isOutput=False)
    inv_in = nc.declare_dram_parameter("invidx", [128, NB], F32, isOutput=False)
    out_d = nc.declare_dram_parameter("out", [S, DM], F32, isOutput=True)
    dbg = {}
    if DEBUG:
        dbg["qhT"] = nc.declare_dram_parameter("d_qhT", [128, 2 * S], F32, isOutput=True)
        dbg["vh"] = nc.declare_dram_parameter("d_vh", [128, NB * DL], F32, isOutput=True)
        dbg["a1"] = nc.declare_dram_parameter("d_a1", [128, NB * S], F32, isOutput=True)
        dbg["sqT"] = nc.declare_dram_parameter("d_sqT", [128, 6144], F32, isOutput=True)
        dbg["wtT"] = nc.declare_dram_parameter("d_wtT", [128, NB * S], F32, isOutput=True)
        dbg["oT"] = nc.declare_dram_parameter("d_oT", [64, HG * S], F32, isOutput=True)
        dbg["den"] = nc.declare_dram_parameter("d_den", [128, NB], F32, isOutput=True)

    with TileContext(nc) as tc:
        with tc.tile_pool(name="persist", bufs=1) as cp, \
             tc.tile_pool(name="ppm", bufs=2, space="PSUM") as ppm, \
             tc.tile_pool(name="ppt", bufs=2, space="PSUM") as ppt:

            mask = cp.tile([128, 4, 512], BF16)
            ident = cp.tile([128, 128], BF16)
            invidx = cp.tile([128, NB], F32)
            wqb = cp.tile([128, 2], F32)
            wkb = cp.tile([128, 2], F32)
            wvb = cp.tile([1, DL], BF16)
            ones1 = cp.tile([1, 128], BF16)
            pTt = cp.tile([128, 2, S], BF16)
            qhT = cp.tile([128, 2, S], BF16)
            khT = cp.tile([128, 2, S], BF16)
            vh = cp.tile([128, NB, DL], BF16)
            oT = cp.tile([128, 2, S], BF16)
            # wc stored per head-pair: wct[:, g, :] = wc rows [g*128:(g+1)*128];
            # loaded up front so the output projection never waits on DMA
            wct = cp.tile([128, 2, S], BF16)

            # ---------------- projections ----------------
            # DMA issue on SP costs ~0.5us per descriptor, so the inputs the
            # first matmuls need go first, split 4-ways for queue parallelism;
            # constants (masks, wc, p) follow.  The v projection runs inside
            # the attention phase (interleaved with head 0's A1/SqT) so its
            # tiles live in a separate pool that outlives the q/k one.
            vp_cm = tc.tile_pool(name="vproj", bufs=1)
            vp = vp_cm.__enter__()
            wvt = vp.tile([128, NB, DL], BF16)
            vTt = vp.tile([128, NB, S], BF16)
            with tc.tile_pool(name="proj", bufs=1) as jp:
                wqt = jp.tile([128, NB, DL], BF16)
                wkt = jp.tile([128, NB, DL], BF16)
                qTt = jp.tile([128, NB, S], BF16)
                kTt = jp.tile([128, NB, S], BF16)
                for wt_, wsrc, xt_, xsrc in ((wqt, wq_in, qTt, qT_in),
                                             (wkt, wk_in, kTt, kT_in),
                                             (wvt, wv_in, vTt, vT_in)):
                    for q4 in range(4):
                        kb = 2 * q4
                        nc.sync.dma_start(
                            out=wt_[:, kb:kb + 2, :],
                            in_=wsrc[kb * 128:(kb + 2) * 128, :].rearrange(
                                "(a p) d -> p a d", p=128))
                        nc.sync.dma_start(
                            out=xt_[:, kb:kb + 2, :],
                            in_=xsrc[kb * 128:(kb + 2) * 128, :].rearrange(
                                "(a p) t -> p a t", p=128))
                    if wt_ is wqt:
                        nc.sync.dma_start(
                            out=pTt[:], in_=pT_in.rearrange("(g p) t -> p g t", p=128))
                        nc.sync.dma_start(out=wqb[:], in_=wqb_in[:])
                        nc.sync.dma_start(out=invidx[:], in_=inv_in[:])
                    elif wt_ is wkt:
                        nc.sync.dma_start(
                            out=mask[:], in_=mask_in.rearrange("r p c -> p r c"))
                        nc.sync.dma_start(out=ident[:], in_=ident_in[:])
                        nc.sync.dma_start(out=wkb[:], in_=wkb_in[:])
                    else:
                        nc.sync.dma_start(out=wvb[:], in_=wvb_in[:])
                        nc.sync.dma_start(out=ones1[:], in_=ones_in[:])
                        nc.sync.dma_start(
                            out=wct[:], in_=wc_in.rearrange("(a p) t -> p a t", p=128))

                # qhT[dm, t] = sum_c wq[c, dm] qT[c, t]  (+bias, * 1/sqrt(D))
                for wt_, xt_, dst, bias_t, scale in (
                    (wqt, qTt, qhT, wqb, NORM_D),
                    (wkt, kTt, khT, wkb, 1.0),
                ):
                    for g in range(2):
                        for n in range(2):
                            ps = ppm.tile([128, 512], F32, tag="mm", name="ps_proj")
                            for kb in range(NB):
                                nc.tensor.matmul(
                                    ps[:], wt_[:, kb, g * 128:(g + 1) * 128],
                                    xt_[:, kb, n * 512:(n + 1) * 512],
                                    start=(kb == 0), stop=(kb == NB - 1))
                            nc.scalar.activation(
                                dst[:, g, n * 512:(n + 1) * 512], ps[:],
                                ACTF.Identity, bias=bias_t[:, g:g + 1], scale=scale)

                if DEBUG:
                    nc.sync.dma_start(out=dbg["qhT"].rearrange("p (a b) -> p a b", a=2),
                                      in_=qhT[:])

            # ---------------- attention (4 heads) ----------------
            # Pair-level software pipeline: A1/SqT for head h+1 are emitted
            # between U(h) and S2(h) so the in-order PE stream always has
            # independent matmuls to run while elementwise chains drain.
            with tc.tile_pool(name="attn", bufs=2) as ap, \
                 tc.tile_pool(name="scr", bufs=2) as sp:
                st = {}

                def gen_a1_sq(h):
                    """Generator: yields after each matmul unit so A1/SqT of
                    head h can be interleaved into head h-1's S2 phase (keeps
                    the in-order PE queue fed while elementwise chains drain).

                    A1 = elu(x)+1 = min(exp(x), 1) + relu(x); exp is safe
                    unclamped (|x| <= ~8 here).  The min runs on idle GPSIMD
                    so PSUM is only held by the exp (ACT) + fused max-add
                    (DVE)."""
                    g, p0 = h // 2, (h % 2) * 64
                    a1 = ap.tile([128, NB, S], BF16, tag="a1", name="a1")
                    sqT = ap.tile([128, 6144], BF16, tag="sq", name="sqT")
                    st[h] = [a1, sqT]
                    for m in range(NB):
                        for c in range(2):
                            ps = ppm.tile([128, 512], F32, tag="a1ps", bufs=2,
                                          name="ps_a1")
                            nc.tensor.matmul(
                                ps[:], qhT[p0:p0 + 64, g, m * 128:(m + 1) * 128],
                                pTt[p0:p0 + 64, g, c * 512:(c + 1) * 512],
                                start=True, stop=True)
                            e = sp.tile([128, 512], F32, tag="e", bufs=4, name="e")
                            nc.scalar.activation(e[:], ps[:], ACTF.Exp)
                            e1 = sp.tile([128, 512], F32, tag="e1", bufs=4, name="e1")
                            nc.gpsimd.tensor_scalar_min(e1[:], e[:], 1.0)
                            nc.vector.scalar_tensor_tensor(
                                a1[:, m, c * 512:(c + 1) * 512], ps[:], 0.0, e1[:],
                                ALU.max, ALU.add)
                            yield
                    for m in range(NB):
                        for n in range(m // 4, 2):
                            ps = ppm.tile([128, 512], F32, tag="mm", name="ps_sq")
                            nc.tensor.matmul(
                                ps[:], khT[p0:p0 + 64, g, m * 128:(m + 1) * 128],
                                qhT[p0:p0 + 64, g, n * 512:(n + 1) * 512],
                                start=True, stop=True)
                            dst = sqT[:, _sq_off(m, n * 512):_sq_off(m, n * 512) + 512]
                            if n == m // 4:
                                nc.vector.tensor_tensor(dst, ps[:], mask[:, m % 4, :], ALU.mult)
                            else:
                                nc.scalar.activation(dst, ps[:], ACTF.Copy)
                            yield
                    if DEBUG and h == 0:
                        nc.sync.dma_start(
                            out=dbg["a1"].rearrange("p (a b) -> p a b", a=NB), in_=a1[:])
                        nc.sync.dma_start(out=dbg["sqT"][:, :], in_=sqT[:])

                def emit_u(h):
                    a1, sqT = st[h]
                    # U row-blocks -> exp(scale*U) -> normalize -> W^T via PE
                    # transpose.  The transposes for block i-1 are emitted
                    # after block i's matmuls so the PE stream never waits on
                    # the exp/normalize chain.
                    wtT = ap.tile([128, NB, S], BF16, tag="wtT", bufs=1, name="wtT")
                    wblks = []

                    def emit_w_transpose(i):
                        wblk = wblks[i]
                        tps = ppt.tile([128, S], BF16, tag="tp", name="tps")
                        for k in range(NB):
                            nc.tensor.transpose(
                                tps[:, k * 128:(k + 1) * 128],
                                wblk[:, k * 128:(k + 1) * 128], ident[:])
                        nc.vector.tensor_copy(
                            wtT[:, :, i * 128:(i + 1) * 128],
                            tps[:].rearrange("p (a b) -> p a b", a=NB))

                    denB = sp.tile([128, NB], F32, tag="denB", name="denB")
                    # descending i: long accumulation groups first, so the
                    # 2-deep PSUM ring never waits on the exp consumer
                    seq = list(range(NB - 1, -1, -1))
                    wblks.extend([None] * NB)
                    for k, i in enumerate(seq):
                        wblk = sp.tile([128, S], BF16, tag="wblk", bufs=NB, name="wblk")
                        wblks[i] = wblk
                        dps = []
                        for c in range(2):
                            ps = ppm.tile([128, 512], F32, tag="mm", name="ps_u")
                            for m in range(i + 1):
                                nc.tensor.matmul(
                                    ps[:], sqT[:, _sq_off(m, i * 128):_sq_off(m, i * 128) + 128],
                                    a1[:, m, c * 512:(c + 1) * 512],
                                    start=(m == 0), stop=(m == i))
                            dp = sp.tile([128, 1], F32, tag="dp", bufs=4, name="dp")
                            nc.scalar.activation(
                                wblk[:, c * 512:(c + 1) * 512], ps[:], ACTF.Exp,
                                scale=invidx[:, i:i + 1], accum_out=dp[:])
                            dps.append(dp)
                        nc.vector.tensor_tensor(denB[:, i:i + 1], dps[0][:], dps[1][:], ALU.add)
                        if k >= 2:
                            emit_w_transpose(seq[k - 2])
                    for k in range(NB - 2, NB):
                        emit_w_transpose(seq[k])
                    if DEBUG and h == 0:
                        nc.sync.dma_start(out=dbg["den"], in_=denB[:])
                        nc.sync.dma_start(
                            out=dbg["wtT"].rearrange("p (a b) -> p a b", a=NB), in_=wtT[:])
                    # 1/denominator as a [1, S] row (t on the free axis) for the
                    # O^T broadcast multiply: reciprocal then a 4KB scatter DMA
                    recden = sp.tile([128, NB], F32, tag="recden", name="recden")
                    nc.vector.reciprocal(recden[:], denB[:])
                    gsc = sp.tile([128, NB], F32, tag="gsc", name="gsc")
                    nc.vector.tensor_tensor(gsc[:], recden[:], invidx[:], ALU.mult)
                    st[h].append(wtT)
                    st[h].append(gsc)

                oNs = {}

                def emit_c(h, inter=None):
                    """out[t-block i] = gsc[t] * (W-blk_i @ C^(i) +
                    tril(W_i A1_i^T) @ vh_i) with the running accumulator
                    C^(i)[j, d] = sum_{s < 128i} A1[s, j] vh[s, d] carried in
                    bf16 (one DVE add per block).  gsc = 1/(den*(t+1)) as a
                    per-partition ACT scale (W was left unnormalized).  Heads
                    h, h+1 share one oN tile (free-axis halves) so a single
                    [128,128] PE transpose per t-block yields the stacked
                    [d, t] layout and the output projection contracts K=128
                    per head-pair."""
                    def pull(k):
                        if inter is not None:
                            for _ in range(k):
                                if next(inter, "done") == "done":
                                    break
                    a1, sqT, wtT, gsc = st.pop(h)
                    d0h = h * 64
                    if h % 2 == 0:
                        oNs[h // 2] = sp.tile([128, NB, 128], BF16, tag="oN",
                                              bufs=2, name="oN")
                    oN = oNs[h // 2]
                    d0 = (h % 2) * 64

                    # a1T strips are produced one block ahead of the S2-diag
                    # matmuls that consume them; the PSUM->SBUF copies
                    # alternate between ACT and DVE to balance engine load
                    a1Ts = {}

                    def emit_a1t(m):
                        a1T = sp.tile([128, NB, 128], BF16, tag="a1T", bufs=4,
                                      name="a1T")
                        a1Ts[m] = a1T
                        tps = ppt.tile([128, S], BF16, tag="tp", name="tps2")
                        for k in range(NB):
                            nc.tensor.transpose(
                                tps[:, k * 128:(k + 1) * 128],
                                a1[:, m, k * 128:(k + 1) * 128], ident[:])
                        tv = tps[:].rearrange("p (a b) -> p a b", a=NB)
                        if m % 2 == 0:
                            nc.scalar.activation(a1T[:], tv, ACTF.Copy)
                        else:
                            nc.vector.tensor_copy(a1T[:], tv)

                    cprev = None
                    emit_a1t(0)
                    for i in range(NB):
                        if i + 1 < NB:
                            emit_a1t(i + 1)
                        a1T = a1Ts.pop(i)
                        # S2-diagonal block: S2dT[s, t] = sum_j A1[s,j] W[t,j]
                        psd = ppm.tile([128, 128], F32, tag="cd", bufs=2,
                                       name="ps_s2d")
                        for k in range(NB):
                            nc.tensor.matmul(
                                psd[:], a1T[:, k, :],
                                wtT[:, k, i * 128:(i + 1) * 128],
                                start=(k == 0), stop=(k == NB - 1))
                        pull(1)
                        s2dT = sp.tile([128, 128], BF16, tag="s2d", bufs=2,
                                       name="s2dT")
                        nc.vector.tensor_tensor(s2dT[:], psd[:],
                                                mask[:, 0, 0:128], ALU.mult)
                        # out-block i: prefix part via C (no dep on the mask),
                        # then the C update, then the diag part — keeps PE fed
                        # while the DVE mask / C-add drain
                        pso = ppm.tile([128, 64], F32, tag="cd", bufs=2,
                                       name="ps_o")
                        if i > 0:
                            for k in range(NB):
                                nc.tensor.matmul(
                                    pso[:], wtT[:, k, i * 128:(i + 1) * 128],
                                    cprev[:, k, :],
                                    start=(k == 0), stop=False)
                        pull(1)
                        psc = None
                        if i + 1 < NB:
                            psc = ppm.tile([128, 512], F32, tag="mm",
                                           name="ps_cu")
                            for k in range(NB):
                                nc.tensor.matmul(
                                    psc[:, k * 64:(k + 1) * 64],
                                    a1[:, i, k * 128:(k + 1) * 128],
                                    vh[:, i, d0h:d0h + 64],
                                    start=True, stop=True)
                        nc.tensor.matmul(pso[:], s2dT[:],
                                         vh[:, i, d0h:d0h + 64],
                                         start=(i == 0), stop=True)
                        nc.scalar.activation(oN[:, i, d0:d0 + 64], pso[:],
                                             ACTF.Copy, scale=gsc[:, i:i + 1])
                        # C update: C^(i+1) = C^(i) + A1_i^T @ vh_i
                        if psc is not None:
                            pull(1)
                            cnew = sp.tile([128, NB, 64], BF16, tag="C",
                                           bufs=3, name="C")
                            pv = psc[:].rearrange("p (a b) -> p a b", a=NB)
                            if cprev is None:
                                nc.vector.tensor_copy(cnew[:], pv)
                            else:
                                nc.vector.tensor_tensor(cnew[:], cprev[:], pv,
                                                        ALU.add)
                            cprev = cnew
                        pull(1)
                        if h == HG - 1:
                            # pipeline the pair-1 oT transpose and the output
                            # projection for t-block i into the last head's
                            # C-phase instead of a serial tail
                            tpo = ppt.tile([128, 128], BF16, tag="tp",
                                           name="tpo")
                            nc.tensor.transpose(tpo[:], oN[:, i, :], ident[:])
                            nc.scalar.activation(
                                oT[:, h // 2, i * 128:(i + 1) * 128], tpo[:],
                                ACTF.Copy)
                            emit_final_tile(i)
                    if h % 2 == 1 and h != HG - 1:
                        oNp = oNs.pop(h // 2)
                        tps = ppt.tile([128, S], BF16, tag="tp", name="tpo")
                        for i in range(NB):
                            nc.tensor.transpose(
                                tps[:, i * 128:(i + 1) * 128], oNp[:, i, :],
                                ident[:])
                        nc.scalar.activation(
                            oT[:, h // 2, :],
                            tps[:].rearrange("p (a b) -> p a b", a=NB),
                            ACTF.Copy)
                    if h == HG - 1:
                        oNs.pop(h // 2)

                def emit_final_tile(i):
                    # out[t-block i, :] = sum_g oT_g^T wc_g (all scales already
                    # folded into oT)
                    for c in range(2):
                        ps = ppm.tile([128, 512], F32, tag="mm", name="ps_fin")
                        for g2 in range(2):
                            nc.tensor.matmul(
                                ps[:], oT[:, g2, i * 128:(i + 1) * 128],
                                wct[:, g2, c * 512:(c + 1) * 512],
                                start=(g2 == 0), stop=(g2 == 1))
                        ot = sp.tile([128, 512], F32, tag="ot", bufs=6, name="ot")
                        if (i + c) % 2 == 0:
                            nc.scalar.activation(ot[:], ps[:], ACTF.Copy)
                        else:
                            nc.vector.tensor_copy(ot[:], ps[:])
                        nc.sync.dma_start(
                            out=out_d[i * 128:(i + 1) * 128, c * 512:(c + 1) * 512],
                            in_=ot[:])

                # vh[s, d] = sum_c vT[c, s] wv[c, d] + wv_b[d], interleaved
                # with head 0's A1/SqT so PE has work while vT streams in
                gen0 = gen_a1_sq(0)
                for m in range(NB):
                    ps = ppm.tile([128, DL], F32, tag="mm", name="ps_vh")
                    for kb in range(NB):
                        nc.tensor.matmul(
                            ps[:], vTt[:, kb, m * 128:(m + 1) * 128], wvt[:, kb, :],
                            start=(kb == 0), stop=False)
                    nc.tensor.matmul(ps[:], ones1[:], wvb[:], start=False, stop=True)
                    nc.scalar.activation(vh[:, m, :], ps[:], ACTF.Copy)
                    for _ in range(3):
                        if next(gen0, "done") == "done":
                            break
                for _ in gen0:
                    pass
                if DEBUG:
                    nc.sync.dma_start(out=dbg["vh"].rearrange("p (a b) -> p a b", a=NB),
                                      in_=vh[:])
                for h in range(HG):
                    emit_u(h)
                    gen = gen_a1_sq(h + 1) if h + 1 < HG else None
                    emit_c(h, inter=gen)
                    if gen is not None:
                        for _ in gen:
                            pass

            if DEBUG:
                nc.sync.dma_start(
                    out=dbg["oT"].rearrange("p (a b) -> p a b", a=HG), in_=oT[:])

            vp_cm.__exit__(None, None, None)

    nc.finalize()
    return nc


_CACHE = {}


def _get_program():
    if "nc" not in _CACHE:
        _CACHE["nc"] = _build_program()
    return _CACHE["nc"]


def _consts():
    if "consts" not in _CACHE:
        p_ = np.arange(128, dtype=np.float32)[:, None]
        c_ = np.arange(512, dtype=np.float32)[None, :]
        mask4 = np.stack(
            [(p_ + 128.0 * r <= c_) for r in range(4)]).astype(NPBF)
        ident = np.eye(128, dtype=np.float32).astype(NPBF)
        blk = np.arange(NB, dtype=np.float32)[None, :]
        invidx = (1.0 / (blk * 128.0 + p_ + 1.0)).astype(np.float32)
        ones1 = np.ones((1, 128), NPBF)
        _CACHE["consts"] = (mask4, ident, invidx, ones1)
    return _CACHE["consts"]


PROFILE = False
LAST_RESULTS = None


def kernel(v, k, q, p, wq_k, wq_b, wk_k, wk_b, wv_k, wv_b, wc_k, wc_b):
    global LAST_RESULTS
    nc = _get_program()
    mask4, ident, invidx, ones1 = _consts()

    qT = [np.ascontiguousarray(q[b].T).astype(NPBF) for b in range(B)]
    kT = [np.ascontiguousarray(k[b].T).astype(NPBF) for b in range(B)]
    vT = [np.ascontiguousarray(v[b].T).astype(NPBF) for b in range(B)]
    pT = [np.ascontiguousarray(p[b].T).astype(NPBF) for b in range(B)]
    wqc = wq_k.astype(NPBF)
    wkc = wk_k.astype(NPBF)
    wvc = wv_k.astype(NPBF)
    wcc = wc_k.astype(NPBF)

    in_maps = []
    for c in range(8):
        b, hg = c // 4, c % 4
        c0 = hg * DL
        wqb = np.ascontiguousarray(
            (wq_b[c0:c0 + DL].reshape(2, 128).T * NORM_D).astype(np.float32))
        wkb = np.ascontiguousarray(wk_b[c0:c0 + DL].reshape(2, 128).T.astype(np.float32))
        in_maps.append({
            "qT": qT[b], "kT": kT[b], "vT": vT[b],
            "pT": np.ascontiguousarray(pT[b][c0:c0 + DL]),
            "wq": np.ascontiguousarray(wqc[:, c0:c0 + DL]),
            "wk": np.ascontiguousarray(wkc[:, c0:c0 + DL]),
            "wv": np.ascontiguousarray(wvc[:, c0:c0 + DL]),
            "wc": np.ascontiguousarray(wcc[c0:c0 + DL, :]),
            "wqb": wqb, "wkb": wkb,
            "wvb": np.ascontiguousarray(wv_b[c0:c0 + DL].reshape(1, DL).astype(NPBF)),
            "ones1": ones1, "mask4": mask4, "ident": ident, "invidx": invidx,
        })

    res = run_bass_kernel_spmd(
        nc, in_maps, core_ids=list(range(8)), trace=PROFILE)
    LAST_RESULTS = res

    out = np.zeros((B, S, DM), np.float32)
    for c in range(8):
        out[c // 4] += res.results[c]["out"]
    out += wc_b[None, None, :].astype(np.float32)
    return out

